# revision 26
# baseline (speedup 1.0000x reference)
"""MoE (8 experts, top-2, shared expert) Trainium2 kernel.

Expert-parallel over 8 NeuronCores, bf16 matmuls (fp32 PSUM accumulate).
The host performs only the dispatch decision (top-2 expert ids -> compact
per-expert token lists) and data layout (every tensor pre-arranged into its
exact SBUF image so each load is one large contiguous DMA); all model FP
math — router logits, gates, expert SwiGLU, shared expert, cross-core
combine — runs on device.

Device program per core (SPMD, identical program, per-core data):
  A:  router logits for compact tokens (matmul) * validity mask -> gates
  B:  ht[I, C] = silu(w1 @ xg) * (w3 @ xg)          (compact tokens)
  C:  y[ct] = gate * (ht.T @ w2) -> eacc (dense compact order, bf16)
  G:  acc[t] = eacc[inv[t]]  (indirect gather with zero-row sentinel for
      tokens not routed to this core; SBUF bounce, write to acc)
  RS: ReduceScatter(add, bf16) over acc -> rst (this core's 256 rows);
      carries ONLY expert contributions, so it overlaps with...
  S:  shared expert (full I) for this core's OWN 256 tokens -> hfin
  F:  out = rst + hfin   (bf16; host upcasts to f32)

(An indirect SCATTER of eacc rows into a pre-zeroed acc models much worse:
the cost model charges a scatter by the full destination tensor size —
5 x 11.6us — so the gather direction is the cheap one.)
"""

import numpy as np

H = 1024          # hidden
I = 1408          # moe intermediate
E = 8             # experts == cores
T = 2048          # tokens (2*1024)
TOPK = 2
C = 576           # compact per-expert token capacity (max observed 540)
ILOC = I // E     # 176 (unused by the kernel; kept for reference)
TSL = T // E      # 256: output token slice per core
KT = H // 128     # 8 contraction tiles over H
IT = I // 128     # 11 tiles over I
NCORES = 8

_BUILD_CACHE = {}


def _cap_geom(cap):
    """Token tiles (offset, len<=128) and B free-dim chunks (<=512)."""
    assert cap % 64 == 0, cap
    tiles = []
    off = 0
    while off < cap:
        L = min(128, cap - off)
        tiles.append((off, L))
        off += L
    chunks = []
    off = 0
    while off < cap:
        L = min(512, cap - off)
        chunks.append((off, L))
        off += L
    return tiles, chunks


def _build(reps=1, use_cc=True, dtype=None, cap=None, prefixes=None):
    import concourse.bacc as bacc
    import concourse.bass as bass
    import concourse.mybir as mybir
    from concourse import tile
    from contextlib import ExitStack

    f32 = mybir.dt.float32
    bf16 = mybir.dt.bfloat16
    i32 = mybir.dt.int32
    AF = mybir.ActivationFunctionType
    MUL = mybir.AluOpType.mult

    cap = cap or C
    tiles, chunks = _cap_geom(cap)
    NT = len(tiles)

    nc = bacc.Bacc("TRN2", target_bir_lowering=False, debug=False,
                   num_devices=NCORES)

    # host-prepared SBUF images, one contiguous DMA each
    xgb = nc.declare_dram_parameter("xgb", [128, KT * cap], bf16,
                                    isOutput=False)
    w1i = nc.declare_dram_parameter("w1i", [128, IT * KT * 128], bf16,
                                    isOutput=False)
    w3i = nc.declare_dram_parameter("w3i", [128, IT * KT * 128], bf16,
                                    isOutput=False)
    w2i = nc.declare_dram_parameter("w2i", [128, IT * H], bf16,
                                    isOutput=False)
    s1i = nc.declare_dram_parameter("s1i", [128, IT * KT * 128], bf16,
                                    isOutput=False)
    s3i = nc.declare_dram_parameter("s3i", [128, IT * KT * 128], bf16,
                                    isOutput=False)
    s2i = nc.declare_dram_parameter("s2i", [128, IT * H], bf16,
                                    isOutput=False)
    xo = nc.declare_dram_parameter("xo", [128, KT * TSL], bf16,
                                   isOutput=False)
    rwe = nc.declare_dram_parameter("rwe", [128, KT * 16], bf16,
                                    isOutput=False)
    invi = nc.declare_dram_parameter("invi", [128, T // 128], i32,
                                     isOutput=False)
    msk = nc.declare_dram_parameter("msk", [128, NT], f32, isOutput=False)
    out = nc.declare_dram_parameter("out", [TSL, H], bf16, isOutput=True)

    acc = nc.dram_tensor("acc", [T, H], bf16)
    # eacc row 0..127: zero sentinel; compact row c lives at 128+c
    eacc = nc.dram_tensor("eacc", [cap + 128, H], bf16)
    rst = nc.dram_tensor("rst", [TSL, H], bf16)
    TT = T // 128
    if prefixes is None:
        prefixes = (cap + 128,) * TT

    with tile.TileContext(nc) as tc, ExitStack() as ctx:
        sres = ctx.enter_context(tc.tile_pool(name="sres", bufs=1))
        wbig = ctx.enter_context(tc.tile_pool(name="wbig", bufs=1))
        work = ctx.enter_context(tc.tile_pool(name="work", bufs=2))
        psA = ctx.enter_context(tc.tile_pool(name="psA", bufs=2, space="PSUM"))
        psB = ctx.enter_context(tc.tile_pool(name="psB", bufs=2, space="PSUM"))
        psY = ctx.enter_context(tc.tile_pool(name="psY", bufs=2, space="PSUM"))
        psL = ctx.enter_context(tc.tile_pool(name="psL", bufs=1, space="PSUM"))

        for _rep in range(reps):
            # ---- loads, in consumption order (B-critical ones first) ----
            xgb_sb = sres.tile([128, KT * cap], bf16, tag="xgb_sb",
                               name="xgb_sb")
            half = (KT // 2) * cap
            nc.sync.dma_start(xgb_sb[:, 0:half], xgb[:, 0:half])
            # expert weights: streamed per-i so B starts after ~0.5 MB
            w1_sb = wbig.tile([128, IT * KT * 128], bf16, tag="wa",
                              name="w1_sb")
            w3_sb = wbig.tile([128, IT * KT * 128], bf16, tag="wb",
                              name="w3_sb")
            sl = slice(0, KT * 128)
            nc.sync.dma_start(w1_sb[:, sl], w1i[:, sl])
            nc.sync.dma_start(w3_sb[:, sl], w3i[:, sl])
            nc.sync.dma_start(xgb_sb[:, half:], xgb[:, half:])
            rwe_sb = sres.tile([128, KT * 16], bf16, tag="rwe_sb",
                               name="rwe_sb")
            nc.sync.dma_start(rwe_sb[:], rwe[:, :])
            msk_sb = sres.tile([128, NT], f32, tag="msk_sb", name="msk_sb")
            nc.sync.dma_start(msk_sb[:], msk[:, :])
            invi_sb = sres.tile([128, TT], i32, tag="invi_sb",
                                name="invi_sb")
            nc.sync.dma_start(invi_sb[:], invi[:, :])
            for i in range(1, IT):
                sl = slice(i * KT * 128, (i + 1) * KT * 128)
                nc.sync.dma_start(w1_sb[:, sl], w1i[:, sl])
                nc.sync.dma_start(w3_sb[:, sl], w3i[:, sl])
            # zero sentinel rows 0..127 for the combine gather
            ztile = work.tile([128, H], bf16, tag="ztile", name="ztile",
                              bufs=1)
            nc.gpsimd.memset(ztile[:], 0.0)
            nc.sync.dma_start(eacc[0:128, :], ztile[:])
            w2_sb = wbig.tile([128, IT * H], bf16, tag="wc", name="w2_sb")
            nc.sync.dma_start(w2_sb[:], w2i[:, :])
            # shared weights in their own buffers: load during B
            s1_sb = wbig.tile([128, IT * KT * 128], bf16, tag="sa",
                              name="s1_sb")
            nc.sync.dma_start(s1_sb[:], s1i[:, :])
            s3_sb = wbig.tile([128, IT * KT * 128], bf16, tag="sb",
                              name="s3_sb")
            nc.sync.dma_start(s3_sb[:], s3i[:, :])
            s2_sb = wbig.tile([128, IT * H], bf16, tag="sc", name="s2_sb")
            nc.sync.dma_start(s2_sb[:], s2i[:, :])
            xo_sb = sres.tile([128, KT * TSL], bf16, tag="xo_sb",
                              name="xo_sb")
            nc.sync.dma_start(xo_sb[:], xo[:, :])

            gates_sb = sres.tile([128, NT], f32, tag="gates_sb",
                                 name="gates_sb")
            ht_sb = sres.tile([128, IT * cap], bf16, tag="ht_sb",
                              name="ht_sb")

            # ---- B: expert ht = silu(w1@xg)*(w3@xg); A after first i ----
            def b_chunk(i, boff, BL):
                psa = psA.tile([128, 512], f32, tag="a", name="psa",
                               space="PSUM")
                psb = psB.tile([128, 512], f32, tag="b", name="psb",
                               space="PSUM")
                for k in range(KT):
                    nc.tensor.matmul(
                        psa[:, :BL],
                        lhsT=w1_sb[:, (i * KT + k) * 128:
                                   (i * KT + k + 1) * 128],
                        rhs=xgb_sb[:, k * cap + boff:k * cap + boff + BL],
                        start=(k == 0), stop=(k == KT - 1))
                for k in range(KT):
                    nc.tensor.matmul(
                        psb[:, :BL],
                        lhsT=w3_sb[:, (i * KT + k) * 128:
                                   (i * KT + k + 1) * 128],
                        rhs=xgb_sb[:, k * cap + boff:k * cap + boff + BL],
                        start=(k == 0), stop=(k == KT - 1))
                sact = work.tile([128, 512], f32, tag="sact", name="sact")
                nc.scalar.activation(sact[:, :BL], psa[:, :BL], AF.Silu)
                nc.vector.tensor_tensor(
                    out=ht_sb[:, i * cap + boff:i * cap + boff + BL],
                    in0=sact[:, :BL], in1=psb[:, :BL], op=MUL)

            def c_tile(ti, toff, TL):
                ysb = work.tile([128, H], bf16, tag="ysb", name="ysb")
                for hh in range(2):
                    psy = psY.tile([128, 512], f32, tag="y", name="psy",
                                   space="PSUM")
                    for i in range(IT):
                        nc.tensor.matmul(
                            psy[:TL, :],
                            lhsT=ht_sb[:, i * cap + toff:i * cap + toff + TL],
                            rhs=w2_sb[:, i * H + hh * 512:
                                      i * H + hh * 512 + 512],
                            start=(i == 0), stop=(i == IT - 1))
                    nc.scalar.activation(
                        ysb[:TL, hh * 512:(hh + 1) * 512],
                        psy[:TL, :], AF.Copy,
                        scale=gates_sb[:TL, ti:ti + 1])
                nc.scalar.dma_start(eacc[128 + toff:128 + toff + TL, :],
                                    ysb[:TL, :])

            for i in range(IT):
                for (boff, BL) in chunks:
                    b_chunk(i, boff, BL)
                if i == 0:
                    # ---- A: router logits -> gates (dense per-token) ----
                    for ti, (toff, TL) in enumerate(tiles):
                        psl = psL.tile([128, 512], f32, tag="l", name="psl",
                                       space="PSUM")
                        for k in range(KT):
                            nc.tensor.matmul(
                                psl[:TL, 0:16],
                                lhsT=xgb_sb[:, k * cap + toff:
                                            k * cap + toff + TL],
                                rhs=rwe_sb[:, k * 16:(k + 1) * 16],
                                start=(k == 0), stop=(k == KT - 1))
                        nc.vector.tensor_tensor(
                            out=gates_sb[:TL, ti:ti + 1],
                            in0=psl[:TL, 0:1], in1=msk_sb[:TL, ti:ti + 1],
                            op=MUL)
            # ---- C: all tiles, in compact-position order ----
            for ti, (toff, TL) in enumerate(tiles):
                c_tile(ti, toff, TL)

            # ---- G: acc[t] = eacc[inv[t]]  (gather with zero sentinel) ----
            # Each gather's source AP is a prefix slice of eacc covering only
            # the rows its tokens can reference (compact order is sorted by
            # token id), so gathers pipeline WITH C instead of after it.
            for tg in range(TT):
                geacc = work.tile([128, H], bf16, tag="geacc",
                                  name="geacc", bufs=10)
                nc.gpsimd.indirect_dma_start(
                    out=geacc[:], out_offset=None,
                    in_=eacc[0:prefixes[tg], :],
                    in_offset=bass.IndirectOffsetOnAxis(
                        ap=invi_sb[:, tg:tg + 1], axis=0))
                nc.sync.dma_start(
                    acc[tg * 128:(tg + 1) * 128, :], geacc[:])

            # ---- RS: expert-only combine; overlaps with S below ----
            if use_cc:
                nc.gpsimd.collective_compute(
                    "ReduceScatter",
                    mybir.AluOpType.add,
                    replica_groups=[list(range(NCORES))],
                    ins=[acc[:, :]],
                    outs=[rst[:, :]],
                )
                src_t = rst
            else:
                src_t = acc

            # ---- S: shared expert, full I, own 256 tokens ----
            hso = sres.tile([128, IT * TSL], bf16, tag="hso", name="hso")
            for i in range(IT):
                psa = psA.tile([128, 512], f32, tag="a", name="psa_s",
                               space="PSUM")
                psb = psB.tile([128, 512], f32, tag="b", name="psb_s",
                               space="PSUM")
                for k in range(KT):
                    nc.tensor.matmul(
                        psa[:, :TSL],
                        lhsT=s1_sb[:, (i * KT + k) * 128:
                                   (i * KT + k + 1) * 128],
                        rhs=xo_sb[:, k * TSL:(k + 1) * TSL],
                        start=(k == 0), stop=(k == KT - 1))
                for k in range(KT):
                    nc.tensor.matmul(
                        psb[:, :TSL],
                        lhsT=s3_sb[:, (i * KT + k) * 128:
                                   (i * KT + k + 1) * 128],
                        rhs=xo_sb[:, k * TSL:(k + 1) * TSL],
                        start=(k == 0), stop=(k == KT - 1))
                sact = work.tile([128, 512], f32, tag="sact", name="sact_s")
                nc.scalar.activation(sact[:, :TSL], psa[:, :TSL], AF.Silu)
                nc.vector.tensor_tensor(
                    out=hso[:, i * TSL:(i + 1) * TSL],
                    in0=sact[:, :TSL], in1=psb[:, :TSL], op=MUL)
            hfin = sres.tile([128, 2 * H], bf16, tag="hfin", name="hfin")
            for t2 in range(2):
                for hh in range(2):
                    psy = psY.tile([128, 512], f32, tag="y", name="psy_s",
                                   space="PSUM")
                    for i in range(IT):
                        nc.tensor.matmul(
                            psy[:],
                            lhsT=hso[:, i * TSL + t2 * 128:
                                     i * TSL + t2 * 128 + 128],
                            rhs=s2_sb[:, i * H + hh * 512:
                                      i * H + hh * 512 + 512],
                            start=(i == 0), stop=(i == IT - 1))
                    # DVE copy keeps Act on the Silu table (no ATL swap)
                    nc.vector.tensor_copy(
                        hfin[:, t2 * H + hh * 512:t2 * H + (hh + 1) * 512],
                        psy[:])

            # ---- F: out = rst + hfin (two halves, pipelined) ----
            for c2 in range(2):
                rsb = work.tile([128, H], bf16, tag="rsb", name="rsb")
                nc.sync.dma_start(rsb[:], src_t[c2 * 128:(c2 + 1) * 128, :])
                obuf = work.tile([128, H], bf16, tag="obuf", name="obuf")
                nc.vector.tensor_add(obuf[:], rsb[:],
                                     hfin[:, c2 * H:(c2 + 1) * H])
                nc.sync.dma_start(out[c2 * 128:(c2 + 1) * 128, :], obuf[:])

    nc.finalize()
    return nc


def _count_max(x2, router_w):
    logits = x2 @ router_w.T
    order = np.argsort(-logits, axis=1, kind="stable")[:, :TOPK]
    return max(int((order == e).any(axis=1).sum()) for e in range(E))


def _dispatch(x2, router_w, cap=None):
    """Host-side sharding decision: per-expert compact token lists."""
    cap = cap or C
    logits = x2 @ router_w.T                      # [T, E] fp32, dispatch only
    order = np.argsort(-logits, axis=1, kind="stable")[:, :TOPK]
    per_core = []
    all_rows = np.arange(T)
    for e in range(E):
        rows = all_rows[(order == e).any(axis=1)]
        ce = len(rows)
        assert ce <= cap, f"expert {e} overflow: {ce} > {cap}"
        unused = np.setdiff1d(all_rows, rows, assume_unique=True)
        pad = np.resize(unused, cap - ce) if cap > ce else unused[:0]
        idx_full = np.concatenate([rows, pad]).astype(np.int32)
        mask = (np.arange(cap) < ce).astype(np.float32)
        per_core.append((idx_full, mask))
    return per_core


def _make_in_maps(x2, router_w, w1, w2, w3, sw1, sw2, sw3, cap=None):
    import ml_dtypes
    bf16 = ml_dtypes.bfloat16

    cap = cap or C
    tiles, _ = _cap_geom(cap)
    NT = len(tiles)
    dispatch = _dispatch(x2, router_w, cap)

    def upimg(w):
        # [I, H] -> [128, IT*KT*128]; img[p, (i*KT+k)*128+m] = w[i*128+m, k*128+p]
        return np.ascontiguousarray(
            np.asarray(w, np.float32).reshape(IT, 128, KT, 128)
            .transpose(3, 0, 2, 1).reshape(128, IT * KT * 128).astype(bf16))

    def dnimg(w):
        # [H, I] -> [128, IT*H]; img[p, i*H+h] = w[h, i*128+p]
        return np.ascontiguousarray(
            np.asarray(w, np.float32).T.reshape(IT, 128, H)
            .transpose(1, 0, 2).reshape(128, IT * H).astype(bf16))

    x2b = x2.astype(bf16)
    # xo[e][p, k*TSL+t] = x2[e*TSL+t, k*128+p]
    xo_all = np.ascontiguousarray(
        x2b.reshape(E, TSL, KT, 128).transpose(0, 3, 2, 1)
        .reshape(E, 128, KT * TSL))
    s1img = upimg(sw1)
    s3img = upimg(sw3)
    s2img = dnimg(sw2)
    rw = np.asarray(router_w, np.float32)

    in_maps = []
    for e in range(E):
        idx_full, mask = dispatch[e]
        xg = x2b[idx_full]                         # [cap, H] bf16
        xgb_img = np.ascontiguousarray(
            xg.reshape(cap, KT, 128).transpose(2, 1, 0)
            .reshape(128, KT * cap))
        rwe_img = np.ascontiguousarray(
            np.repeat(rw[e].reshape(KT, 128).T[:, :, None], 16, axis=2)
            .reshape(128, KT * 16).astype(bf16))
        mpad = np.zeros(NT * 128, np.float32)
        mpad[:cap] = mask
        ce = int(mask.sum())
        inv = np.zeros(T, dtype=np.int32)          # sentinel -> zero row 0
        inv[idx_full[:ce]] = 128 + np.arange(ce, dtype=np.int32)
        in_maps.append({
            "xgb": xgb_img,
            "w1i": upimg(w1[e]),
            "w3i": upimg(w3[e]),
            "w2i": dnimg(w2[e]),
            "s1i": s1img,
            "s3i": s3img,
            "s2i": s2img,
            "xo": xo_all[e],
            "rwe": rwe_img,
            "invi": np.ascontiguousarray(inv.reshape(T // 128, 128).T),
            "msk": np.ascontiguousarray(mpad.reshape(NT, 128).T),
        })
    return in_maps


def _prefixes(x2, router_w, cap):
    """Per token-tile eacc prefix (max over cores, for the shared SPMD
    program): gather tg only reads eacc rows < P[tg]."""
    logits = x2 @ router_w.T
    order = np.argsort(-logits, axis=1, kind="stable")[:, :TOPK]
    TT = T // 128
    P = np.full(TT, 128, dtype=np.int64)
    for e in range(E):
        rows = np.arange(T)[(order == e).any(axis=1)]
        cum = np.searchsorted(rows, (np.arange(TT) + 1) * 128)
        P = np.maximum(P, 128 + cum)
    return tuple(int(min(p, cap + 128)) for p in P)


def _nc_for(x2, router_w, cap=None):
    """The exact module kernel() will run for these inputs (cached)."""
    cap = cap or C
    cmax = _count_max(x2, router_w)
    if cmax > cap:  # unlikely re-routed inputs: rebuild with larger capacity
        cap = -((-cmax) // 64) * 64
    prefixes = _prefixes(x2, router_w, cap)
    key = (1, cap, prefixes)
    if key not in _BUILD_CACHE:
        _BUILD_CACHE[key] = _build(1, cap=cap, prefixes=prefixes)
    return _BUILD_CACHE[key], cap


def kernel(x, router_w, w1, w2, w3, sw1, sw2, sw3):
    from concourse.bass_utils import run_bass_kernel_spmd

    in_dtype = x.dtype
    x2 = np.ascontiguousarray(x.reshape(T, H), dtype=np.float32)
    router_w = np.asarray(router_w, dtype=np.float32)
    nc, cap = _nc_for(x2, router_w)

    in_maps = _make_in_maps(x2, router_w, w1, w2, w3, sw1, sw2, sw3, cap)
    res = run_bass_kernel_spmd(nc, in_maps, list(range(NCORES)))
    out = np.concatenate(
        [np.asarray(res.results[i]["out"], dtype=np.float32)
         for i in range(NCORES)], axis=0)
    return out.reshape(x.shape).astype(in_dtype)
